# revision 18
# baseline (speedup 1.0000x reference)
"""Trainium2 Bass kernel for CausalNCMomentumAttention (linear attention,
causal + non-causal branches).

Shapes (hardcoded): N=2, L=8192, H=8, E=M=64, fp32 in/out.
Sharding: 8 cores; core i handles batch n = i//4, heads 2*(i%4)..+1.

The PE's HAM clock gate only counts full-row (128-contraction) matmuls
as busy: any 64-row matmul stream runs at the cold 1.2 GHz clock forever
(measured).  So EVERY matmul here contracts over 128 partitions, with
the two heads packed by block-diagonal weights and col-tiled outputs
((128,64) mode keeps HAM warm; (64,x) does not):

  qt2 [128,L]    stacked heads: rows 0:64 = Qf_h0^T, 64:128 = Qf_h1^T
  ktp [128,2,L]  ktp[:,h] has Kf_h^T in rows h*64:.., ZEROS elsewhere
  ks  [128,c,128] chunk-major [Kf_h0 | Kf_h1];  v2m same for V*mask

  D:     2 col-tiled MMs -> d[0:64,0:64]=D_h0, d[64:,64:]=D_h1; the
         start=True bank clear zeroes the off-diagonal blocks, so the
         fp32 running state SP and its bf16 snapshots SS[c] are
         block-diagonal by construction.
  at:    at_h[s,l] = ktp[:,h,cb]^T @ qt2[cb]  (zero rows kill the other
         head), masked to l>=s on DVE into bf16 at_sb.
  inter: ONE MM/chunk: SS[c] (block-diag) @ qt2 -> [128,128] stacked nums
  intra: 2 col-tiled MMs: v2m[:,c,h-block]^T @ at_h accumulates rows
         h*64:(h+1)*64 of the same PSUM tile.
  nc:    SfinBD (block-diag final state) @ qt2, N=512 blocks.

Outputs are unnormalized bf16 numerators ([128,NCH,128] causal,
[128,L] non-causal, rows = stacked (h,m)); the host applies the feature
map (elu+1, fp32) and computes both denominators (fp32 cumsum/einsum -
more accurate than the old on-device bf16 path) and the final divide.

Engine split: DVE masks at (2-chunk PSUM banks) + fp32 state adds;
GpSimd casts state snapshots; Scalar evacuates vc (4-chunk banks) and
half the nc tiles (DVE the other half).
"""

import sys
import numpy as np

if "/opt/trn_rl_repo" not in sys.path:
    sys.path.insert(0, "/opt/trn_rl_repo")

import concourse.bass as bass
import concourse.bacc as bacc
import concourse.tile as tile
from concourse import mybir
from concourse.bass_utils import run_bass_kernel_spmd

F32 = mybir.dt.float32
BF16 = mybir.dt.bfloat16
ALU = mybir.AluOpType

N, L, H, E, M = 2, 8192, 8, 64, 64
C = 128
NCH = L // C            # 64 chunks
G = 8                   # chunks per group
NG = NCH // G           # 8 groups
EPS = 1e-6


def emit(tc, nc, qt2, kt, ks, v2m, o_vc, o_nc):
    with (
        tc.tile_pool(name="const", bufs=1) as const,
        tc.tile_pool(name="big", bufs=1) as big,
    ):
        wub = const.tile([C, C], BF16)          # warm-up weights only
        nc.vector.memset(wub, 0.0)
        iot = const.tile([C, C], mybir.dt.int32)
        nc.gpsimd.iota(iot, pattern=[[1, C]], base=0, channel_multiplier=-1)
        tri = const.tile([C, C], BF16)          # tri[s,l] = (l >= s)
        nc.vector.tensor_scalar(tri, iot, 0, None, ALU.is_ge)

        qt2_t = big.tile([C, L], BF16)
        ktp_t = big.tile([C, 2, L], BF16)
        ks_t = big.tile([C, NCH, C], BF16)
        v2m_t = big.tile([C, NCH, C], BF16)
        SS = big.tile([C, NCH, C], BF16)        # block-diag S_c snapshots
        SP = big.tile([C, 2, C], F32)           # ping-pong fp32 state
        SfinBD = big.tile([C, C], BF16)
        nc_sb = big.tile([C, L], BF16)
        nc.vector.memset(SP, 0.0)
        nc.gpsimd.memset(SS[:, 0], 0.0)
        # zero halves of ktp (other-head rows must kill the contraction)
        nc.gpsimd.memset(ktp_t[E:C, 0, :], 0.0)
        nc.gpsimd.memset(ktp_t[0:E, 1, :], 0.0)

        def load_kv(a_lo, a_hi):
            sl_a = slice(a_lo, a_hi)
            nc.sync.dma_start(out=ks_t[:, sl_a], in_=ks[:, sl_a])
            nc.sync.dma_start(out=v2m_t[:, sl_a], in_=v2m[:, sl_a])

        def load_q(a_lo, a_hi):
            sl_l = slice(a_lo * C, a_hi * C)
            nc.sync.dma_start(out=ktp_t[0:E, 0, sl_l], in_=kt[:, 0, sl_l])
            nc.sync.dma_start(out=ktp_t[E:C, 1, sl_l], in_=kt[:, 1, sl_l])
            nc.sync.dma_start(out=qt2_t[:, sl_l], in_=qt2[:, sl_l])

        def load_slice(a_lo, a_hi):
            load_kv(a_lo, a_hi)
            load_q(a_lo, a_hi)

        load_kv(0, G // 2)
        load_kv(G // 2, 2 * G)
        load_q(0, G)
        load_kv(2 * G, 3 * G)
        load_q(G, 3 * G)

        with (
            tc.tile_pool(name="atsb", bufs=6) as atsb_pool,
            tc.tile_pool(name="ovb", bufs=3) as ovb_pool,
            tc.tile_pool(name="d_ps", bufs=1, space="PSUM") as d_pool,
            tc.tile_pool(name="at_ps", bufs=2, space="PSUM") as at_pool,
            tc.tile_pool(name="vc_ps", bufs=3, space="PSUM") as vc_pool,
        ):
            # persistent ping-pong D tiles: the col-tiled head MMs only
            # clear/write their own diagonal block, so the off-blocks
            # must be zeroed once and never touched again
            d_a = d_pool.tile([C, C], F32, tag="da")
            d_b = d_pool.tile([C, C], F32, tag="db")
            d_ab = [d_a, d_b]
            nc.vector.memset(d_ab[0], 0.0)
            nc.vector.memset(d_ab[1], 0.0)

            # HAM warm-up: dense full-mode MMs while the prologue DMA
            # runs (the PE is otherwise idle and would start at 1.2 GHz).
            # Re-warm again once the DMA-starved early window has passed
            # (it==2): from cold, only a >=3.4us dense burst un-throttles.
            def warm_burst(n, tag):
                wu = at_pool.tile([C, 2, 2, C], F32, tag="at")
                wuv = wu.rearrange("p a b c -> p (a b c)")
                for _ in range(n):
                    nc.tensor.matmul(wuv[:, 0:128], lhsT=wub, rhs=wub,
                                     start=True, stop=True,
                                     skip_group_check=True)

            warm_burst(64, "wu0")
            wudrip = d_pool.tile([C, 256], F32, tag="wudrip")

            def drip(n=1):
                # N=256 keeps the HAM busy-fraction high per weight load
                for _ in range(n):
                    nc.tensor.matmul(wudrip[0:64, :], lhsT=wub[:, 0:64],
                                     rhs=qt2_t[:, 0:256], start=True,
                                     stop=True, skip_group_check=True)
            for it in range(NG + 1):
                if 1 <= it <= 3:
                    load_slice((2 * it + 1) * G,
                               min((2 * it + 3) * G, NCH))
                if it == 2:
                    warm_burst(48, "wu2")
                g = it - 1
                g0 = g * G

                def at_bank(p2):
                    # 2-chunk at PSUM bank for group g + DVE mask evac
                    at_ps = at_pool.tile([C, 2, 2, C], F32, tag="at")
                    for j in range(2):
                        c = g0 + 2 * p2 + j
                        cb = slice(c * C, (c + 1) * C)
                        for h in range(2):
                            nc.tensor.matmul(
                                at_ps[:, j, h, :], lhsT=ktp_t[:, h, cb],
                                rhs=qt2_t[:, cb],
                                start=(j == 0 and h == 0),
                                stop=(j == 1 and h == 1),
                                skip_group_check=True)
                    at_sb = atsb_pool.tile([C, 2, 2, C], BF16, tag="atsb")
                    nc.vector.tensor_tensor(
                        at_sb, at_ps,
                        tri[:, None, None, :].broadcast_to([C, 2, 2, C]),
                        ALU.mult)
                    at_tiles.append(at_sb)

                # ---- D phase (group `it`) with at-banks of group g
                # interleaved to keep the PE stream dense ----
                at_tiles = []
                if it < NG:
                    for cc in range(G):
                        c = it * G + cc
                        d = d_ab[c % 2]
                        nc.tensor.matmul(
                            d[0:E, 0:E], lhsT=ks_t[:, c, 0:E],
                            rhs=v2m_t[:, c, 0:E], start=True, stop=True,
                            skip_group_check=True)
                        nc.tensor.matmul(
                            d[E:C, E:C], lhsT=ks_t[:, c, E:C],
                            rhs=v2m_t[:, c, E:C], start=True, stop=True,
                            skip_group_check=True)
                        pp, pn = c % 2, (c + 1) % 2
                        nc.vector.tensor_tensor(
                            SP[:, pn], d, SP[:, pp], ALU.add)
                        dst = SfinBD if c == NCH - 1 else SS[:, c + 1]
                        if c % 2 == 0:
                            nc.gpsimd.tensor_copy(dst, SP[:, pn])
                        else:
                            nc.scalar.copy(out=dst, in_=SP[:, pn])
                        drip()
                        if it >= 1 and cc % 2 == 0:
                            at_bank(cc // 2)
                else:
                    for p2 in range(G // 2):
                        at_bank(p2)

                if it == 0:
                    continue

                # ---- inter phase: one block-diag MM per chunk ----
                vc_tiles = []
                for q4 in range(G // 4):
                    vc_ps = vc_pool.tile([C, 4, C], F32, tag="vc")
                    for jj in range(4):
                        c = g0 + 4 * q4 + jj
                        cb = slice(c * C, (c + 1) * C)
                        nc.tensor.matmul(
                            vc_ps[:, jj, :], lhsT=SS[:, c],
                            rhs=qt2_t[:, cb], start=(jj == 0), stop=False,
                            skip_group_check=True)
                        drip()
                    vc_tiles.append(vc_ps)

                # ---- intra phase: col-tiled accumulate + evac ----
                ovb = ovb_pool.tile([C, 4, C], BF16, tag="ovb")
                ovb2 = ovb_pool.tile([C, 4, C], BF16, tag="ovb")
                for q4 in range(G // 4):
                    vc_ps = vc_tiles[q4]
                    for jj in range(4):
                        c = g0 + 4 * q4 + jj
                        p2, j = (4 * q4 + jj) // 2, jj % 2
                        for h in range(2):
                            nc.tensor.matmul(
                                vc_ps[h * E:(h + 1) * E, jj, :],
                                lhsT=v2m_t[:, c, h * E:(h + 1) * E],
                                rhs=at_tiles[p2][:, j, h, :],
                                start=False, stop=(jj == 3 and h == 1),
                                skip_group_check=True)
                    dst = ovb if q4 == 0 else ovb2
                    nc.scalar.copy(out=dst, in_=vc_ps)
                nc.sync.dma_start(out=o_vc[:, g0:g0 + 4], in_=ovb)
                nc.sync.dma_start(out=o_vc[:, g0 + 4:g0 + G], in_=ovb2)

                # ---- nc phase (last iter): SfinBD @ qt2, N=512 ----
                if it == NG:
                    for blk in range(L // 512):
                        lb = slice(blk * 512, (blk + 1) * 512)
                        if blk % 2 == 0:
                            ncp = at_pool.tile([C, 2, 2, C], F32, tag="at")
                            ncv = ncp.rearrange("p a b c -> p (a b c)")
                        else:
                            ncp = vc_pool.tile([C, 4, C], F32, tag="vc")
                            ncv = ncp.rearrange("p a c -> p (a c)")
                        nc.tensor.matmul(
                            ncv, lhsT=SfinBD, rhs=qt2_t[:, lb],
                            start=True, stop=True, skip_group_check=True)
                        if blk % 2 == 0:
                            nc.scalar.copy(out=nc_sb[:, lb], in_=ncv)
                        else:
                            nc.vector.tensor_copy(nc_sb[:, lb], ncv)
                        if blk % 2 == 1:
                            lq = slice((blk - 1) * 512, (blk + 1) * 512)
                            nc.sync.dma_start(
                                out=o_nc[:, lq], in_=nc_sb[:, lq])


def build():
    nc = bacc.Bacc("TRN2", target_bir_lowering=False, debug=False)
    qt2 = nc.dram_tensor("qt2", [C, L], BF16, kind="ExternalInput").ap()
    kt = nc.dram_tensor("kt", [E, 2, L], BF16, kind="ExternalInput").ap()
    ks = nc.dram_tensor("ks", [C, NCH, C], BF16, kind="ExternalInput").ap()
    v2m = nc.dram_tensor("v2m", [C, NCH, C], BF16, kind="ExternalInput").ap()
    o_vc = nc.dram_tensor("o_vc", [C, NCH, C], BF16,
                          kind="ExternalOutput").ap()
    o_nc = nc.dram_tensor("o_nc", [C, L], BF16, kind="ExternalOutput").ap()
    with tile.TileContext(nc) as tc:
        emit(tc, nc, qt2, kt, ks, v2m, o_vc, o_nc)
    nc.compile()
    return nc


_NC = None
_last_in_maps = None


def _get_nc():
    global _NC
    if _NC is None:
        _NC = build()
    return _NC


def _bf16(x):
    import ml_dtypes
    return np.ascontiguousarray(x, dtype=np.float32).astype(ml_dtypes.bfloat16)


def _feat(x):
    # elu(x) + 1 in fp32: exp(min(x,0)) + relu(x)
    return np.exp(np.minimum(x, 0.0)) + np.maximum(x, 0.0)


def kernel(queries, keys, values, key_mask):
    global _last_in_maps
    nc = _get_nc()
    queries = np.asarray(queries, dtype=np.float32)
    keys = np.asarray(keys, dtype=np.float32)
    values = np.asarray(values, dtype=np.float32)
    key_mask = np.asarray(key_mask, dtype=np.float32)

    Qf = _feat(queries)
    Kf = _feat(keys) * key_mask[:, :, None, None]
    Vm = values * key_mask[:, :, None, None]
    # fp32 denominators on host (exact reference math)
    denc = np.einsum('nlhe,nlhe->nlh', Qf, np.cumsum(Kf, axis=1)) + EPS
    dennc = np.einsum('nlhe,nhe->nlh', Qf, Kf.sum(axis=1)) + EPS

    in_maps = []
    for i in range(8):
        n, h0 = i // 4, 2 * (i % 4)
        qh = Qf[n, :, h0:h0 + 2, :]                   # [L, 2, 64]
        kh = Kf[n, :, h0:h0 + 2, :]
        vh = Vm[n, :, h0:h0 + 2, :]
        qs = qh.transpose(1, 2, 0).reshape(C, L)      # stacked heads
        in_maps.append({
            "qt2": _bf16(qs),
            "kt": _bf16(kh.transpose(2, 1, 0)),
            "ks": _bf16(kh.reshape(NCH, C, C).transpose(1, 0, 2)),
            "v2m": _bf16(vh.reshape(NCH, C, C).transpose(1, 0, 2)),
        })
    _last_in_maps = in_maps
    res = run_bass_kernel_spmd(nc, in_maps, core_ids=list(range(8)))
    V = np.empty((N, L, H, M), np.float32)
    Vc = np.empty((N, L, H, M), np.float32)
    for i in range(8):
        n, h0 = i // 4, 2 * (i % 4)
        ovc = res.results[i]["o_vc"].astype(np.float32)   # [128, NCH, 128]
        onc = res.results[i]["o_nc"].astype(np.float32)   # [128, L]
        num_c = ovc.transpose(1, 2, 0).reshape(L, C)      # [l, (h m)]
        num_n = onc.T                                     # [l, (h m)]
        for h in range(2):
            Vc[n, :, h0 + h, :] = (num_c[:, h * E:(h + 1) * E]
                                   / denc[n, :, h0 + h, None])
            V[n, :, h0 + h, :] = (num_n[:, h * E:(h + 1) * E]
                                  / dennc[n, :, h0 + h, None])
    return (V, Vc)


# revision 19
# speedup vs baseline: 1.0857x; 1.0857x over previous
"""Trainium2 Bass kernel for CausalNCMomentumAttention (linear attention,
causal + non-causal branches).

Shapes (hardcoded): N=2, L=8192, H=8, E=M=64, fp32 in/out.
Sharding: 8 cores; core i handles batch n = i//4, heads 2*(i%4)..+1.

The PE's HAM clock gate only counts full-row (128-contraction) matmuls
as busy: any 64-row matmul stream runs at the cold 1.2 GHz clock forever
(measured).  So EVERY matmul here contracts over 128 partitions, with
the two heads packed by block-diagonal weights and col-tiled outputs
((128,64) mode keeps HAM warm; (64,x) does not):

  qt2 [128,L]    stacked heads: rows 0:64 = Qf_h0^T, 64:128 = Qf_h1^T
  ktp [128,2,L]  ktp[:,h] has Kf_h^T in rows h*64:.., ZEROS elsewhere
  ks  [128,c,128] chunk-major [Kf_h0 | Kf_h1];  v2m same for V*mask

  D:     2 col-tiled MMs -> d[0:64,0:64]=D_h0, d[64:,64:]=D_h1; the
         start=True bank clear zeroes the off-diagonal blocks, so the
         fp32 running state SP and its bf16 snapshots SS[c] are
         block-diagonal by construction.
  at:    at_h[s,l] = ktp[:,h,cb]^T @ qt2[cb]  (zero rows kill the other
         head), masked to l>=s on DVE into bf16 at_sb.
  inter: ONE MM/chunk: SS[c] (block-diag) @ qt2 -> [128,128] stacked nums
  intra: 2 col-tiled MMs: v2m[:,c,h-block]^T @ at_h accumulates rows
         h*64:(h+1)*64 of the same PSUM tile.
  nc:    SfinBD (block-diag final state) @ qt2, N=512 blocks.

Outputs are unnormalized bf16 numerators ([128,NCH,128] causal,
[128,L] non-causal, rows = stacked (h,m)); the host applies the feature
map (elu+1, fp32) and computes both denominators (fp32 cumsum/einsum -
more accurate than the old on-device bf16 path) and the final divide.

Engine split: DVE masks at (2-chunk PSUM banks) + fp32 state adds;
GpSimd casts state snapshots; Scalar evacuates vc (4-chunk banks) and
half the nc tiles (DVE the other half).
"""

import sys
import numpy as np

if "/opt/trn_rl_repo" not in sys.path:
    sys.path.insert(0, "/opt/trn_rl_repo")

import concourse.bass as bass
import concourse.bacc as bacc
import concourse.tile as tile
from concourse import mybir
from concourse.bass_utils import run_bass_kernel_spmd

F32 = mybir.dt.float32
BF16 = mybir.dt.bfloat16
ALU = mybir.AluOpType

N, L, H, E, M = 2, 8192, 8, 64, 64
C = 128
NCH = L // C            # 64 chunks
G = 8                   # chunks per group
NG = NCH // G           # 8 groups
EPS = 1e-6


def emit(tc, nc, qt2, kt, ks, v2m, o_vc, o_nc):
    with (
        tc.tile_pool(name="const", bufs=1) as const,
        tc.tile_pool(name="big", bufs=1) as big,
    ):
        wub = const.tile([C, C], BF16)          # warm-up weights only
        nc.vector.memset(wub, 0.0)
        iot = const.tile([C, C], mybir.dt.int32)
        nc.gpsimd.iota(iot, pattern=[[1, C]], base=0, channel_multiplier=-1)
        tri = const.tile([C, C], BF16)          # tri[s,l] = (l >= s)
        nc.vector.tensor_scalar(tri, iot, 0, None, ALU.is_ge)

        qt2_t = big.tile([C, L], BF16)
        ktp_t = big.tile([C, 2, L], BF16)
        ks_t = big.tile([C, NCH, C], BF16)
        v2m_t = big.tile([C, NCH, C], BF16)
        SS = big.tile([C, NCH, C], BF16)        # block-diag S_c snapshots
        SP = big.tile([C, 2, C], F32)           # ping-pong fp32 state
        SfinBD = big.tile([C, C], BF16)
        nc_sb = big.tile([C, L], BF16)
        nc.vector.memset(SP, 0.0)
        nc.gpsimd.memset(SS[:, 0], 0.0)
        # zero halves of ktp (other-head rows must kill the contraction)
        nc.gpsimd.memset(ktp_t[E:C, 0, :], 0.0)
        nc.gpsimd.memset(ktp_t[0:E, 1, :], 0.0)

        def load_kv(a_lo, a_hi):
            sl_a = slice(a_lo, a_hi)
            nc.sync.dma_start(out=ks_t[:, sl_a], in_=ks[:, sl_a])
            nc.sync.dma_start(out=v2m_t[:, sl_a], in_=v2m[:, sl_a])

        def load_q(a_lo, a_hi):
            sl_l = slice(a_lo * C, a_hi * C)
            nc.sync.dma_start(out=ktp_t[0:E, 0, sl_l], in_=kt[:, 0, sl_l])
            nc.sync.dma_start(out=ktp_t[E:C, 1, sl_l], in_=kt[:, 1, sl_l])
            nc.sync.dma_start(out=qt2_t[:, sl_l], in_=qt2[:, sl_l])

        def load_slice(a_lo, a_hi):
            load_kv(a_lo, a_hi)
            load_q(a_lo, a_hi)

        load_kv(0, G // 2)
        load_kv(G // 2, 2 * G)
        load_q(0, G)
        load_kv(2 * G, 3 * G)
        load_q(G, 3 * G)

        with (
            tc.tile_pool(name="atsb", bufs=6) as atsb_pool,
            tc.tile_pool(name="ovb", bufs=3) as ovb_pool,
            tc.tile_pool(name="d_ps", bufs=1, space="PSUM") as d_pool,
            tc.tile_pool(name="at_ps", bufs=2, space="PSUM") as at_pool,
            tc.tile_pool(name="vc_ps", bufs=3, space="PSUM") as vc_pool,
        ):
            # persistent ping-pong D tiles: the col-tiled head MMs only
            # clear/write their own diagonal block, so the off-blocks
            # must be zeroed once and never touched again
            d_a = d_pool.tile([C, C], F32, tag="da")
            d_b = d_pool.tile([C, C], F32, tag="db")
            d_ab = [d_a, d_b]
            nc.vector.memset(d_ab[0], 0.0)
            nc.vector.memset(d_ab[1], 0.0)

            # HAM warm-up: dense full-mode MMs while the prologue DMA
            # runs (the PE is otherwise idle and would start at 1.2 GHz).
            # Re-warm again once the DMA-starved early window has passed
            # (it==2): from cold, only a >=3.4us dense burst un-throttles.
            def warm_burst(n, tag):
                wu = at_pool.tile([C, 2, 2, C], F32, tag="at")
                wuv = wu.rearrange("p a b c -> p (a b c)")
                for _ in range(n):
                    nc.tensor.matmul(wuv[:, 0:128], lhsT=wub, rhs=wub,
                                     start=True, stop=True,
                                     skip_group_check=True)

            warm_burst(64, "wu0")
            wudrip = d_pool.tile([C, C], F32, tag="wudrip")

            def drip(n=1):
                for _ in range(n):
                    nc.tensor.matmul(wudrip[0:64, :], lhsT=wub[:, 0:64],
                                     rhs=wub, start=True, stop=True,
                                     skip_group_check=True)
            for it in range(NG + 1):
                if 1 <= it <= 3:
                    load_slice((2 * it + 1) * G,
                               min((2 * it + 3) * G, NCH))
                if it == 2:
                    warm_burst(48, "wu2")
                g = it - 1
                g0 = g * G

                def at_bank(p2):
                    # 2-chunk at PSUM bank for group g + DVE mask evac
                    at_ps = at_pool.tile([C, 2, 2, C], F32, tag="at")
                    for j in range(2):
                        c = g0 + 2 * p2 + j
                        cb = slice(c * C, (c + 1) * C)
                        for h in range(2):
                            nc.tensor.matmul(
                                at_ps[:, j, h, :], lhsT=ktp_t[:, h, cb],
                                rhs=qt2_t[:, cb],
                                start=(j == 0 and h == 0),
                                stop=(j == 1 and h == 1),
                                skip_group_check=True)
                    at_sb = atsb_pool.tile([C, 2, 2, C], BF16, tag="atsb")
                    nc.vector.tensor_tensor(
                        at_sb, at_ps,
                        tri[:, None, None, :].broadcast_to([C, 2, 2, C]),
                        ALU.mult)
                    at_tiles.append(at_sb)

                # ---- D phase (group `it`) with at-banks of group g
                # interleaved to keep the PE stream dense ----
                at_tiles = []
                if it < NG:
                    for cc in range(G):
                        c = it * G + cc
                        d = d_ab[c % 2]
                        nc.tensor.matmul(
                            d[0:E, 0:E], lhsT=ks_t[:, c, 0:E],
                            rhs=v2m_t[:, c, 0:E], start=True, stop=True,
                            skip_group_check=True)
                        nc.tensor.matmul(
                            d[E:C, E:C], lhsT=ks_t[:, c, E:C],
                            rhs=v2m_t[:, c, E:C], start=True, stop=True,
                            skip_group_check=True)
                        pp, pn = c % 2, (c + 1) % 2
                        nc.vector.tensor_tensor(
                            SP[:, pn], d, SP[:, pp], ALU.add)
                        dst = SfinBD if c == NCH - 1 else SS[:, c + 1]
                        if c % 2 == 0:
                            nc.gpsimd.tensor_copy(dst, SP[:, pn])
                        else:
                            nc.scalar.copy(out=dst, in_=SP[:, pn])
                        drip()
                        if it >= 1 and cc % 2 == 0:
                            at_bank(cc // 2)
                else:
                    for p2 in range(G // 2):
                        at_bank(p2)

                if it == 0:
                    continue

                # ---- inter phase: one block-diag MM per chunk ----
                vc_tiles = []
                for q4 in range(G // 4):
                    vc_ps = vc_pool.tile([C, 4, C], F32, tag="vc")
                    for jj in range(4):
                        c = g0 + 4 * q4 + jj
                        cb = slice(c * C, (c + 1) * C)
                        nc.tensor.matmul(
                            vc_ps[:, jj, :], lhsT=SS[:, c],
                            rhs=qt2_t[:, cb], start=(jj == 0), stop=False,
                            skip_group_check=True)
                        drip()
                    vc_tiles.append(vc_ps)

                # ---- intra phase: col-tiled accumulate + evac ----
                ovb = ovb_pool.tile([C, 4, C], BF16, tag="ovb")
                ovb2 = ovb_pool.tile([C, 4, C], BF16, tag="ovb")
                for q4 in range(G // 4):
                    vc_ps = vc_tiles[q4]
                    for jj in range(4):
                        c = g0 + 4 * q4 + jj
                        p2, j = (4 * q4 + jj) // 2, jj % 2
                        for h in range(2):
                            nc.tensor.matmul(
                                vc_ps[h * E:(h + 1) * E, jj, :],
                                lhsT=v2m_t[:, c, h * E:(h + 1) * E],
                                rhs=at_tiles[p2][:, j, h, :],
                                start=False, stop=(jj == 3 and h == 1),
                                skip_group_check=True)
                    dst = ovb if q4 == 0 else ovb2
                    nc.scalar.copy(out=dst, in_=vc_ps)
                nc.sync.dma_start(out=o_vc[:, g0:g0 + 4], in_=ovb)
                nc.sync.dma_start(out=o_vc[:, g0 + 4:g0 + G], in_=ovb2)

                # ---- nc phase (last iter): SfinBD @ qt2, N=512 ----
                if it == NG:
                    for blk in range(L // 512):
                        lb = slice(blk * 512, (blk + 1) * 512)
                        if blk % 2 == 0:
                            ncp = at_pool.tile([C, 2, 2, C], F32, tag="at")
                            ncv = ncp.rearrange("p a b c -> p (a b c)")
                        else:
                            ncp = vc_pool.tile([C, 4, C], F32, tag="vc")
                            ncv = ncp.rearrange("p a c -> p (a c)")
                        nc.tensor.matmul(
                            ncv, lhsT=SfinBD, rhs=qt2_t[:, lb],
                            start=True, stop=True, skip_group_check=True)
                        if blk % 2 == 0:
                            nc.scalar.copy(out=nc_sb[:, lb], in_=ncv)
                        else:
                            nc.vector.tensor_copy(nc_sb[:, lb], ncv)
                        if blk % 2 == 1:
                            lq = slice((blk - 1) * 512, (blk + 1) * 512)
                            nc.sync.dma_start(
                                out=o_nc[:, lq], in_=nc_sb[:, lq])


def build():
    nc = bacc.Bacc("TRN2", target_bir_lowering=False, debug=False)
    qt2 = nc.dram_tensor("qt2", [C, L], BF16, kind="ExternalInput").ap()
    kt = nc.dram_tensor("kt", [E, 2, L], BF16, kind="ExternalInput").ap()
    ks = nc.dram_tensor("ks", [C, NCH, C], BF16, kind="ExternalInput").ap()
    v2m = nc.dram_tensor("v2m", [C, NCH, C], BF16, kind="ExternalInput").ap()
    o_vc = nc.dram_tensor("o_vc", [C, NCH, C], BF16,
                          kind="ExternalOutput").ap()
    o_nc = nc.dram_tensor("o_nc", [C, L], BF16, kind="ExternalOutput").ap()
    with tile.TileContext(nc) as tc:
        emit(tc, nc, qt2, kt, ks, v2m, o_vc, o_nc)
    nc.compile()
    return nc


_NC = None
_last_in_maps = None


def _get_nc():
    global _NC
    if _NC is None:
        _NC = build()
    return _NC


def _bf16(x):
    import ml_dtypes
    return np.ascontiguousarray(x, dtype=np.float32).astype(ml_dtypes.bfloat16)


def _feat(x):
    # elu(x) + 1 in fp32: exp(min(x,0)) + relu(x)
    return np.exp(np.minimum(x, 0.0)) + np.maximum(x, 0.0)


def kernel(queries, keys, values, key_mask):
    global _last_in_maps
    nc = _get_nc()
    queries = np.asarray(queries, dtype=np.float32)
    keys = np.asarray(keys, dtype=np.float32)
    values = np.asarray(values, dtype=np.float32)
    key_mask = np.asarray(key_mask, dtype=np.float32)

    Qf = _feat(queries)
    Kf = _feat(keys) * key_mask[:, :, None, None]
    Vm = values * key_mask[:, :, None, None]
    # fp32 denominators on host (exact reference math)
    denc = np.einsum('nlhe,nlhe->nlh', Qf, np.cumsum(Kf, axis=1)) + EPS
    dennc = np.einsum('nlhe,nhe->nlh', Qf, Kf.sum(axis=1)) + EPS

    in_maps = []
    for i in range(8):
        n, h0 = i // 4, 2 * (i % 4)
        qh = Qf[n, :, h0:h0 + 2, :]                   # [L, 2, 64]
        kh = Kf[n, :, h0:h0 + 2, :]
        vh = Vm[n, :, h0:h0 + 2, :]
        qs = qh.transpose(1, 2, 0).reshape(C, L)      # stacked heads
        in_maps.append({
            "qt2": _bf16(qs),
            "kt": _bf16(kh.transpose(2, 1, 0)),
            "ks": _bf16(kh.reshape(NCH, C, C).transpose(1, 0, 2)),
            "v2m": _bf16(vh.reshape(NCH, C, C).transpose(1, 0, 2)),
        })
    _last_in_maps = in_maps
    res = run_bass_kernel_spmd(nc, in_maps, core_ids=list(range(8)))
    V = np.empty((N, L, H, M), np.float32)
    Vc = np.empty((N, L, H, M), np.float32)
    for i in range(8):
        n, h0 = i // 4, 2 * (i % 4)
        ovc = res.results[i]["o_vc"].astype(np.float32)   # [128, NCH, 128]
        onc = res.results[i]["o_nc"].astype(np.float32)   # [128, L]
        num_c = ovc.transpose(1, 2, 0).reshape(L, C)      # [l, (h m)]
        num_n = onc.T                                     # [l, (h m)]
        for h in range(2):
            Vc[n, :, h0 + h, :] = (num_c[:, h * E:(h + 1) * E]
                                   / denc[n, :, h0 + h, None])
            V[n, :, h0 + h, :] = (num_n[:, h * E:(h + 1) * E]
                                  / dennc[n, :, h0 + h, None])
    return (V, Vc)


# revision 20
# speedup vs baseline: 1.1119x; 1.0242x over previous
"""Trainium2 Bass kernel for CausalNCMomentumAttention (linear attention,
causal + non-causal branches).

Shapes (hardcoded): N=2, L=8192, H=8, E=M=64, fp32 in/out.
Sharding: 8 cores; core i handles batch n = i//4, heads 2*(i%4)..+1.

The PE's HAM clock gate only counts full-row (128-contraction) matmuls
as busy: any 64-row matmul stream runs at the cold 1.2 GHz clock forever
(measured).  So EVERY matmul contracts over 128 partitions, with the
two heads packed by block-diagonal weights and col-tiled outputs
((128,64)/(128,128) keep HAM warm; (64,x) does not).  Dummy "drip"
matmuls and two warm-up bursts hold the clock gate open through sparse
stream sections.

Work is tiled in 256-row SUPERCHUNKS (two 128-row PE chunks c0,c1):

  qt2 [128,L]    stacked heads: rows 0:64 = Qf_h0^T, 64:128 = Qf_h1^T
  ktp [128,2,L]  ktp[:,h] has Kf_h^T in rows h*64:.., ZEROS elsewhere
  ks/v2m [128,NCH,128]  chunk-major [h0 | h1] blocks of Kf and V*mask

  D:     4 col-tiled MMs accumulate the 256-row state update into the
         diagonal blocks of one persistent PSUM tile (off-blocks zeroed
         once); ONE fp32 SP add + ONE bf16 block-diag snapshot SS[sc]
         per superchunk.
  at:    per head: ktp[c0]^T @ qt2[c0:c2] (N=256: the cross block
         [s in c0, l in c1] IS c1's inter term w.r.t. c0) and
         ktp[c1]^T @ qt2[c1] (N=128); ONE DVE evac per superchunk with
         the composite mask [tri | ones | tri] -> bf16 at_sb.
  inter: ONE MM per superchunk: SS[sc] (block-diag) @ qt2 (N=256).
  intra: per head: v2m[c0,h]^T @ at_sb[h,0:256] (N=256, covers diag c0
         AND the cross term) + v2m[c1,h]^T @ at_sb[h,256:384] into the
         l-c1 half; col-tiled into rows h*64:(h+1)*64.
  nc:    SfinBD (block-diag final state) @ qt2, N=512 blocks.

Outputs are unnormalized bf16 numerators ([128,NCH,128] causal,
[128,L] non-causal, rows = stacked (h,m)); the host applies the feature
map (elu+1, fp32), computes both denominators (fp32 cumsum/einsum) and
the final divide.
"""

import sys
import numpy as np

if "/opt/trn_rl_repo" not in sys.path:
    sys.path.insert(0, "/opt/trn_rl_repo")

import concourse.bass as bass
import concourse.bacc as bacc
import concourse.tile as tile
from concourse import mybir
from concourse.bass_utils import run_bass_kernel_spmd

F32 = mybir.dt.float32
BF16 = mybir.dt.bfloat16
ALU = mybir.AluOpType

N, L, H, E, M = 2, 8192, 8, 64, 64
C = 128
NCH = L // C            # 64 chunks
NSC = NCH // 2          # 32 superchunks
G = 8                   # chunks per group
SG = G // 2             # superchunks per group
NG = NCH // G           # 8 groups
EPS = 1e-6


def emit(tc, nc, qt2, kt, ks, v2m, o_vc, o_nc):
    with (
        tc.tile_pool(name="const", bufs=1) as const,
        tc.tile_pool(name="big", bufs=1) as big,
    ):
        wub = const.tile([C, C], BF16)          # warm-up weights only
        nc.vector.memset(wub, 0.0)
        iot = const.tile([C, C], mybir.dt.int32)
        nc.gpsimd.iota(iot, pattern=[[1, C]], base=0, channel_multiplier=-1)
        atmask = const.tile([C, 3, C], BF16)    # [tri | ones | tri]
        nc.vector.tensor_scalar(atmask[:, 0, :], iot, 0, None, ALU.is_ge)
        nc.vector.memset(atmask[:, 1, :], 1.0)
        nc.vector.tensor_copy(atmask[:, 2, :], atmask[:, 0, :])

        qt2_t = big.tile([C, L], BF16)
        ktp_t = big.tile([C, 2, L], BF16)
        ks_t = big.tile([C, NCH, C], BF16)
        v2m_t = big.tile([C, NCH, C], BF16)
        SS = big.tile([C, NSC, C], BF16)        # block-diag S snapshots
        SP = big.tile([C, 2, C], F32)           # ping-pong fp32 state
        SfinBD = big.tile([C, C], BF16)
        nc_sb = big.tile([C, L], BF16)
        nc.vector.memset(SP, 0.0)
        nc.gpsimd.memset(SS[:, 0], 0.0)
        # zero halves of ktp (other-head rows must kill the contraction)
        nc.gpsimd.memset(ktp_t[E:C, 0, :], 0.0)
        nc.gpsimd.memset(ktp_t[0:E, 1, :], 0.0)

        def load_kv(a_lo, a_hi):
            sl_a = slice(a_lo, a_hi)
            nc.sync.dma_start(out=ks_t[:, sl_a], in_=ks[:, sl_a])
            nc.sync.dma_start(out=v2m_t[:, sl_a], in_=v2m[:, sl_a])

        def load_q(a_lo, a_hi):
            sl_l = slice(a_lo * C, a_hi * C)
            nc.sync.dma_start(out=ktp_t[0:E, 0, sl_l], in_=kt[:, 0, sl_l])
            nc.sync.dma_start(out=ktp_t[E:C, 1, sl_l], in_=kt[:, 1, sl_l])
            nc.sync.dma_start(out=qt2_t[:, sl_l], in_=qt2[:, sl_l])

        load_kv(0, G // 2)
        load_kv(G // 2, 2 * G)
        load_q(0, G)
        load_kv(2 * G, 3 * G)
        load_q(G, 3 * G)

        with (
            tc.tile_pool(name="atsb", bufs=5) as atsb_pool,
            tc.tile_pool(name="ovb", bufs=3) as ovb_pool,
            tc.tile_pool(name="d_ps", bufs=1, space="PSUM") as d_pool,
            tc.tile_pool(name="at_ps", bufs=2, space="PSUM") as at_pool,
            tc.tile_pool(name="vc_ps", bufs=2, space="PSUM") as vc_pool,
        ):
            # persistent ping-pong D tile: the col-tiled head MMs only
            # clear/write their own diagonal blocks, so the off-blocks
            # are zeroed once here and never touched again
            dd = d_pool.tile([C, 2, C], F32, tag="dd")
            nc.vector.memset(dd, 0.0)
            wudrip = d_pool.tile([C, C], F32, tag="wudrip")

            def drip(n=1):
                for _ in range(n):
                    nc.tensor.matmul(wudrip[0:64, :], lhsT=wub[:, 0:64],
                                     rhs=wub, start=True, stop=True,
                                     skip_group_check=True)

            # HAM warm-up: dense full-mode MMs while the prologue DMA
            # runs; re-warm once the DMA-starved early window has passed
            def warm_burst(n):
                wu = at_pool.tile([C, 2, 512], F32, tag="at")
                for _ in range(n):
                    nc.tensor.matmul(wu[:, 0, 0:128], lhsT=wub, rhs=wub,
                                     start=True, stop=True,
                                     skip_group_check=True)

            warm_burst(64)

            for it in range(NG + 1):
                if 1 <= it <= 3:
                    a_lo = (2 * it + 1) * G
                    a_hi = min(a_lo + 2 * G, NCH)
                    load_kv(a_lo, a_hi)
                    load_q(a_lo, a_hi)
                if it == 2:
                    warm_burst(48)
                g = it - 1
                g0 = g * G

                at_tiles = []

                def at_bank(p2):
                    # superchunk at tile: per head N=256 (diag c0 +
                    # cross c0->c1) then N=128 (diag c1); one DVE evac
                    c0 = g0 + 2 * p2
                    cb0 = slice(c0 * C, (c0 + 1) * C)
                    cb2 = slice(c0 * C, (c0 + 2) * C)
                    cb1 = slice((c0 + 1) * C, (c0 + 2) * C)
                    at_ps = at_pool.tile([C, 2, 512], F32, tag="at")
                    for h in range(2):
                        nc.tensor.matmul(
                            at_ps[:, h, 0:256], lhsT=ktp_t[:, h, cb0],
                            rhs=qt2_t[:, cb2],
                            start=True, stop=False, skip_group_check=True)
                        nc.tensor.matmul(
                            at_ps[:, h, 256:384], lhsT=ktp_t[:, h, cb1],
                            rhs=qt2_t[:, cb1],
                            start=True, stop=(h == 1),
                            skip_group_check=True)
                    at_sb = atsb_pool.tile([C, 2, 3, C], BF16, tag="atsb")
                    nc.vector.tensor_tensor(
                        at_sb,
                        at_ps[:, :, 0:384].rearrange(
                            "p h (a c) -> p h a c", a=3),
                        atmask[:, None, :, :].broadcast_to([C, 2, 3, C]),
                        ALU.mult)
                    at_tiles.append(at_sb)

                # ---- D phase (group `it`, per superchunk) with group
                # g's at-banks interleaved to keep the PE stream dense
                if it < NG:
                    for scc in range(SG):
                        c0 = it * G + 2 * scc
                        sc = it * SG + scc
                        sp_p, sp_n = sc % 2, (sc + 1) % 2
                        d = dd[:, sc % 2, :]
                        for j in range(2):
                            c = c0 + j
                            nc.tensor.matmul(
                                d[0:E, 0:E], lhsT=ks_t[:, c, 0:E],
                                rhs=v2m_t[:, c, 0:E], start=(j == 0),
                                stop=(j == 1), skip_group_check=True)
                            nc.tensor.matmul(
                                d[E:C, E:C], lhsT=ks_t[:, c, E:C],
                                rhs=v2m_t[:, c, E:C], start=(j == 0),
                                stop=(j == 1), skip_group_check=True)
                        nc.vector.tensor_tensor(
                            SP[:, sp_n], d, SP[:, sp_p], ALU.add)
                        dst = SfinBD if sc == NSC - 1 else SS[:, sc + 1]
                        if scc % 2 == 0:
                            nc.gpsimd.tensor_copy(dst, SP[:, sp_n])
                        else:
                            nc.scalar.copy(out=dst, in_=SP[:, sp_n])
                        drip()
                        if it >= 1:
                            at_bank(scc)
                else:
                    for p2 in range(SG):
                        at_bank(p2)
                        drip()

                if it == 0:
                    continue

                # ---- inter + intra per superchunk of group g ----
                ovb = ovb_pool.tile([C, 4, C], BF16, tag="ovb")
                ovb2 = ovb_pool.tile([C, 4, C], BF16, tag="ovb")
                for p2 in range(SG):
                    c0 = g0 + 2 * p2
                    sc = g * SG + p2
                    cb2 = slice(c0 * C, (c0 + 2) * C)
                    vc_ps = vc_pool.tile([C, 2, C], F32, tag="vc")
                    vcv = vc_ps.rearrange("p a c -> p (a c)")
                    nc.tensor.matmul(
                        vcv, lhsT=SS[:, sc], rhs=qt2_t[:, cb2],
                        start=True, stop=False, skip_group_check=True)
                    for h in range(2):
                        hb = slice(h * E, (h + 1) * E)
                        nc.tensor.matmul(
                            vcv[hb, :], lhsT=v2m_t[:, c0, hb],
                            rhs=at_tiles[p2][:, h, 0:2, :].rearrange(
                                "p a c -> p (a c)"),
                            start=False, stop=False, skip_group_check=True)
                        nc.tensor.matmul(
                            vcv[hb, 128:256], lhsT=v2m_t[:, c0 + 1, hb],
                            rhs=at_tiles[p2][:, h, 2, :],
                            start=False, stop=(h == 1),
                            skip_group_check=True)
                    dst = ovb if p2 < 2 else ovb2
                    nc.scalar.copy(
                        out=dst[:, 2 * (p2 % 2):2 * (p2 % 2) + 2, :],
                        in_=vc_ps)
                nc.sync.dma_start(out=o_vc[:, g0:g0 + 4], in_=ovb)
                nc.sync.dma_start(out=o_vc[:, g0 + 4:g0 + G], in_=ovb2)

                # ---- nc phase (last iter): SfinBD @ qt2, N=512, two
                # blocks per 2-bank at tile ----
                if it == NG:
                    for bp in range(L // 1024):
                        ncp = at_pool.tile([C, 2, 512], F32, tag="at")
                        ncf = ncp.rearrange("p a c -> p (a c)")
                        for half in range(2):
                            blk = 2 * bp + half
                            lb = slice(blk * 512, (blk + 1) * 512)
                            ncv = ncf[:, half * 512:(half + 1) * 512]
                            nc.tensor.matmul(
                                ncv, lhsT=SfinBD, rhs=qt2_t[:, lb],
                                start=True, stop=True,
                                skip_group_check=True)
                            if half == 0:
                                nc.scalar.copy(out=nc_sb[:, lb], in_=ncv)
                            else:
                                nc.vector.tensor_copy(nc_sb[:, lb], ncv)
                        lq = slice(bp * 1024, (bp + 1) * 1024)
                        nc.sync.dma_start(out=o_nc[:, lq], in_=nc_sb[:, lq])


def build():
    nc = bacc.Bacc("TRN2", target_bir_lowering=False, debug=False)
    qt2 = nc.dram_tensor("qt2", [C, L], BF16, kind="ExternalInput").ap()
    kt = nc.dram_tensor("kt", [E, 2, L], BF16, kind="ExternalInput").ap()
    ks = nc.dram_tensor("ks", [C, NCH, C], BF16, kind="ExternalInput").ap()
    v2m = nc.dram_tensor("v2m", [C, NCH, C], BF16, kind="ExternalInput").ap()
    o_vc = nc.dram_tensor("o_vc", [C, NCH, C], BF16,
                          kind="ExternalOutput").ap()
    o_nc = nc.dram_tensor("o_nc", [C, L], BF16, kind="ExternalOutput").ap()
    with tile.TileContext(nc) as tc:
        emit(tc, nc, qt2, kt, ks, v2m, o_vc, o_nc)
    nc.compile()
    return nc


_NC = None
_last_in_maps = None


def _get_nc():
    global _NC
    if _NC is None:
        _NC = build()
    return _NC


def _bf16(x):
    import ml_dtypes
    return np.ascontiguousarray(x, dtype=np.float32).astype(ml_dtypes.bfloat16)


def _feat(x):
    # elu(x) + 1 in fp32: exp(min(x,0)) + relu(x)
    return np.exp(np.minimum(x, 0.0)) + np.maximum(x, 0.0)


def kernel(queries, keys, values, key_mask):
    global _last_in_maps
    nc = _get_nc()
    queries = np.asarray(queries, dtype=np.float32)
    keys = np.asarray(keys, dtype=np.float32)
    values = np.asarray(values, dtype=np.float32)
    key_mask = np.asarray(key_mask, dtype=np.float32)

    Qf = _feat(queries)
    Kf = _feat(keys) * key_mask[:, :, None, None]
    Vm = values * key_mask[:, :, None, None]
    denc = np.einsum('nlhe,nlhe->nlh', Qf, np.cumsum(Kf, axis=1)) + EPS
    dennc = np.einsum('nlhe,nhe->nlh', Qf, Kf.sum(axis=1)) + EPS

    in_maps = []
    for i in range(8):
        n, h0 = i // 4, 2 * (i % 4)
        qh = Qf[n, :, h0:h0 + 2, :]                   # [L, 2, 64]
        kh = Kf[n, :, h0:h0 + 2, :]
        vh = Vm[n, :, h0:h0 + 2, :]
        qs = qh.transpose(1, 2, 0).reshape(C, L)      # stacked heads
        in_maps.append({
            "qt2": _bf16(qs),
            "kt": _bf16(kh.transpose(2, 1, 0)),
            "ks": _bf16(kh.reshape(NCH, C, C).transpose(1, 0, 2)),
            "v2m": _bf16(vh.reshape(NCH, C, C).transpose(1, 0, 2)),
        })
    _last_in_maps = in_maps
    res = run_bass_kernel_spmd(nc, in_maps, core_ids=list(range(8)))
    V = np.empty((N, L, H, M), np.float32)
    Vc = np.empty((N, L, H, M), np.float32)
    for i in range(8):
        n, h0 = i // 4, 2 * (i % 4)
        ovc = res.results[i]["o_vc"].astype(np.float32)   # [128, NCH, 128]
        onc = res.results[i]["o_nc"].astype(np.float32)   # [128, L]
        num_c = ovc.transpose(1, 2, 0).reshape(L, C)      # [l, (h m)]
        num_n = onc.T                                     # [l, (h m)]
        for h in range(2):
            Vc[n, :, h0 + h, :] = (num_c[:, h * E:(h + 1) * E]
                                   / denc[n, :, h0 + h, None])
            V[n, :, h0 + h, :] = (num_n[:, h * E:(h + 1) * E]
                                  / dennc[n, :, h0 + h, None])
    return (V, Vc)


# revision 21
# speedup vs baseline: 1.1335x; 1.0195x over previous
"""Trainium2 Bass kernel for CausalNCMomentumAttention (linear attention,
causal + non-causal branches).

Shapes (hardcoded): N=2, L=8192, H=8, E=M=64, fp32 in/out.
Sharding: 8 cores; core i handles batch n = i//4, heads 2*(i%4)..+1.

The PE's HAM clock gate only counts full-row (128-contraction) matmuls
as busy: any 64-row matmul stream runs at the cold 1.2 GHz clock forever
(measured).  So EVERY matmul contracts over 128 partitions, with the
two heads packed by block-diagonal weights and col-tiled outputs
((128,64)/(128,128) keep HAM warm; (64,x) does not).  Dummy "drip"
matmuls and two warm-up bursts hold the clock gate open through sparse
stream sections.

Work is tiled in 256-row SUPERCHUNKS (two 128-row PE chunks c0,c1):

  qt2 [128,L]    stacked heads: rows 0:64 = Qf_h0^T, 64:128 = Qf_h1^T
  ktp [128,2,L]  ktp[:,h] has Kf_h^T in rows h*64:.., ZEROS elsewhere
  ks/v2m [128,NCH,128]  chunk-major [h0 | h1] blocks of Kf and V*mask

  D:     4 col-tiled MMs accumulate the 256-row state update into the
         diagonal blocks of one persistent PSUM tile (off-blocks zeroed
         once); ONE fp32 SP add + ONE bf16 block-diag snapshot SS[sc]
         per superchunk.
  at:    per head: ktp[c0]^T @ qt2[c0:c2] (N=256: the cross block
         [s in c0, l in c1] IS c1's inter term w.r.t. c0) and
         ktp[c1]^T @ qt2[c1] (N=128); ONE DVE evac per superchunk with
         the composite mask [tri | ones | tri] -> bf16 at_sb.
  inter: ONE MM per superchunk: SS[sc] (block-diag) @ qt2 (N=256).
  intra: per head: v2m[c0,h]^T @ at_sb[h,0:256] (N=256, covers diag c0
         AND the cross term) + v2m[c1,h]^T @ at_sb[h,256:384] into the
         l-c1 half; col-tiled into rows h*64:(h+1)*64.
  nc:    SfinBD (block-diag final state) @ qt2, N=512 blocks.

Outputs are unnormalized bf16 numerators ([128,NCH,128] causal,
[128,L] non-causal, rows = stacked (h,m)); the host applies the feature
map (elu+1, fp32), computes both denominators (fp32 cumsum/einsum) and
the final divide.
"""

import sys
import numpy as np

if "/opt/trn_rl_repo" not in sys.path:
    sys.path.insert(0, "/opt/trn_rl_repo")

import concourse.bass as bass
import concourse.bacc as bacc
import concourse.tile as tile
from concourse import mybir
from concourse.bass_utils import run_bass_kernel_spmd

F32 = mybir.dt.float32
BF16 = mybir.dt.bfloat16
ALU = mybir.AluOpType

N, L, H, E, M = 2, 8192, 8, 64, 64
C = 128
NCH = L // C            # 64 chunks
NSC = NCH // 2          # 32 superchunks
G = 8                   # chunks per group
SG = G // 2             # superchunks per group
NG = NCH // G           # 8 groups
EPS = 1e-6


def emit(tc, nc, qt2, kt, ks, v2m, o_vc, o_nc):
    with (
        tc.tile_pool(name="const", bufs=1) as const,
        tc.tile_pool(name="big", bufs=1) as big,
    ):
        wub = const.tile([C, C], BF16)          # warm-up weights only
        nc.vector.memset(wub, 0.0)
        iot = const.tile([C, C], mybir.dt.int32)
        nc.gpsimd.iota(iot, pattern=[[1, C]], base=0, channel_multiplier=-1)
        atmask = const.tile([C, 3, C], BF16)    # [tri | ones | tri]
        nc.vector.tensor_scalar(atmask[:, 0, :], iot, 0, None, ALU.is_ge)
        nc.vector.memset(atmask[:, 1, :], 1.0)
        nc.vector.tensor_copy(atmask[:, 2, :], atmask[:, 0, :])

        qt2_t = big.tile([C, L], BF16)
        ktp_t = big.tile([C, 2, L], BF16)
        ks_t = big.tile([C, NCH, C], BF16)
        v2m_t = big.tile([C, NCH, C], BF16)
        SS = big.tile([C, NSC, C], BF16)        # block-diag S snapshots
        SP = big.tile([C, 2, C], F32)           # ping-pong fp32 state
        SfinBD = big.tile([C, C], BF16)
        nc_sb = big.tile([C, L], BF16)
        nc.vector.memset(SP, 0.0)
        nc.gpsimd.memset(SS[:, 0], 0.0)
        # zero halves of ktp (other-head rows must kill the contraction)
        nc.gpsimd.memset(ktp_t[E:C, 0, :], 0.0)
        nc.gpsimd.memset(ktp_t[0:E, 1, :], 0.0)

        def load_kv(a_lo, a_hi):
            sl_a = slice(a_lo, a_hi)
            nc.sync.dma_start(out=ks_t[:, sl_a], in_=ks[:, sl_a])
            nc.sync.dma_start(out=v2m_t[:, sl_a], in_=v2m[:, sl_a])

        def load_q(a_lo, a_hi):
            sl_l = slice(a_lo * C, a_hi * C)
            nc.sync.dma_start(out=ktp_t[0:E, 0, sl_l], in_=kt[:, 0, sl_l])
            nc.sync.dma_start(out=ktp_t[E:C, 1, sl_l], in_=kt[:, 1, sl_l])
            nc.sync.dma_start(out=qt2_t[:, sl_l], in_=qt2[:, sl_l])

        load_kv(0, G)
        load_q(0, G)
        load_kv(G, 3 * G)
        load_q(G, 3 * G)

        with (
            tc.tile_pool(name="atsb", bufs=5) as atsb_pool,
            tc.tile_pool(name="ovb", bufs=3) as ovb_pool,
            tc.tile_pool(name="d_ps", bufs=1, space="PSUM") as d_pool,
            tc.tile_pool(name="at_ps", bufs=2, space="PSUM") as at_pool,
            tc.tile_pool(name="vc_ps", bufs=2, space="PSUM") as vc_pool,
        ):
            # persistent ping-pong D tile: the col-tiled head MMs only
            # clear/write their own diagonal blocks, so the off-blocks
            # are zeroed once here and never touched again
            dd = d_pool.tile([C, 2, C], F32, tag="dd")
            nc.vector.memset(dd, 0.0)
            wudrip = d_pool.tile([C, C], F32, tag="wudrip")

            def drip(n=1):
                for _ in range(n):
                    nc.tensor.matmul(wudrip[0:64, :], lhsT=wub[:, 0:64],
                                     rhs=wub, start=True, stop=True,
                                     skip_group_check=True)

            # HAM warm-up: dense full-mode MMs while the prologue DMA
            # runs; re-warm once the DMA-starved early window has passed
            def warm_burst(n):
                wu = at_pool.tile([C, 2, 512], F32, tag="at")
                for _ in range(n):
                    nc.tensor.matmul(wu[:, 0, 0:128], lhsT=wub, rhs=wub,
                                     start=True, stop=True,
                                     skip_group_check=True)

            warm_burst(64)

            for it in range(NG + 1):
                if 1 <= it <= 3:
                    a_lo = (2 * it + 1) * G
                    a_hi = min(a_lo + 2 * G, NCH)
                    load_kv(a_lo, a_hi)
                    load_q(a_lo, a_hi)
                if it == 2:
                    warm_burst(48)
                g = it - 1
                g0 = g * G

                at_tiles = []

                def at_bank(p2):
                    # superchunk at tile: per head N=256 (diag c0 +
                    # cross c0->c1) then N=128 (diag c1); one DVE evac
                    c0 = g0 + 2 * p2
                    cb0 = slice(c0 * C, (c0 + 1) * C)
                    cb2 = slice(c0 * C, (c0 + 2) * C)
                    cb1 = slice((c0 + 1) * C, (c0 + 2) * C)
                    at_ps = at_pool.tile([C, 2, 512], F32, tag="at")
                    for h in range(2):
                        nc.tensor.matmul(
                            at_ps[:, h, 0:256], lhsT=ktp_t[:, h, cb0],
                            rhs=qt2_t[:, cb2],
                            start=True, stop=False, skip_group_check=True)
                        nc.tensor.matmul(
                            at_ps[:, h, 256:384], lhsT=ktp_t[:, h, cb1],
                            rhs=qt2_t[:, cb1],
                            start=True, stop=(h == 1),
                            skip_group_check=True)
                    at_sb = atsb_pool.tile([C, 2, 3, C], BF16, tag="atsb")
                    nc.vector.tensor_tensor(
                        at_sb,
                        at_ps[:, :, 0:384].rearrange(
                            "p h (a c) -> p h a c", a=3),
                        atmask[:, None, :, :].broadcast_to([C, 2, 3, C]),
                        ALU.mult)
                    at_tiles.append(at_sb)

                # ---- D phase (group `it`, per superchunk) with group
                # g's at-banks interleaved to keep the PE stream dense
                if it < NG:
                    for scc in range(SG):
                        c0 = it * G + 2 * scc
                        sc = it * SG + scc
                        sp_p, sp_n = sc % 2, (sc + 1) % 2
                        d = dd[:, sc % 2, :]
                        for j in range(2):
                            c = c0 + j
                            nc.tensor.matmul(
                                d[0:E, 0:E], lhsT=ks_t[:, c, 0:E],
                                rhs=v2m_t[:, c, 0:E], start=(j == 0),
                                stop=(j == 1), skip_group_check=True)
                            nc.tensor.matmul(
                                d[E:C, E:C], lhsT=ks_t[:, c, E:C],
                                rhs=v2m_t[:, c, E:C], start=(j == 0),
                                stop=(j == 1), skip_group_check=True)
                        nc.vector.tensor_tensor(
                            SP[:, sp_n], d, SP[:, sp_p], ALU.add)
                        dst = SfinBD if sc == NSC - 1 else SS[:, sc + 1]
                        if scc % 2 == 0:
                            nc.gpsimd.tensor_copy(dst, SP[:, sp_n])
                        else:
                            nc.scalar.copy(out=dst, in_=SP[:, sp_n])
                        drip()
                        if it >= 1:
                            at_bank(scc)
                else:
                    for p2 in range(SG):
                        at_bank(p2)
                        drip()

                if it == 0:
                    continue

                # ---- inter + intra per superchunk of group g ----
                ovb = ovb_pool.tile([C, 4, C], BF16, tag="ovb")
                ovb2 = ovb_pool.tile([C, 4, C], BF16, tag="ovb")
                for p2 in range(SG):
                    c0 = g0 + 2 * p2
                    sc = g * SG + p2
                    cb2 = slice(c0 * C, (c0 + 2) * C)
                    vc_ps = vc_pool.tile([C, 2, C], F32, tag="vc")
                    vcv = vc_ps.rearrange("p a c -> p (a c)")
                    nc.tensor.matmul(
                        vcv, lhsT=SS[:, sc], rhs=qt2_t[:, cb2],
                        start=True, stop=False, skip_group_check=True)
                    for h in range(2):
                        hb = slice(h * E, (h + 1) * E)
                        nc.tensor.matmul(
                            vcv[hb, :], lhsT=v2m_t[:, c0, hb],
                            rhs=at_tiles[p2][:, h, 0:2, :].rearrange(
                                "p a c -> p (a c)"),
                            start=False, stop=False, skip_group_check=True)
                        nc.tensor.matmul(
                            vcv[hb, 128:256], lhsT=v2m_t[:, c0 + 1, hb],
                            rhs=at_tiles[p2][:, h, 2, :],
                            start=False, stop=(h == 1),
                            skip_group_check=True)
                    dst = ovb if p2 < 2 else ovb2
                    nc.scalar.copy(
                        out=dst[:, 2 * (p2 % 2):2 * (p2 % 2) + 2, :],
                        in_=vc_ps)
                nc.sync.dma_start(out=o_vc[:, g0:g0 + 4], in_=ovb)
                nc.sync.dma_start(out=o_vc[:, g0 + 4:g0 + G], in_=ovb2)

                # ---- nc phase (last iter): SfinBD @ qt2, N=512, two
                # blocks per 2-bank at tile ----
                if it == NG:
                    for bp in range(L // 1024):
                        ncp = at_pool.tile([C, 2, 512], F32, tag="at")
                        ncf = ncp.rearrange("p a c -> p (a c)")
                        for half in range(2):
                            blk = 2 * bp + half
                            lb = slice(blk * 512, (blk + 1) * 512)
                            ncv = ncf[:, half * 512:(half + 1) * 512]
                            nc.tensor.matmul(
                                ncv, lhsT=SfinBD, rhs=qt2_t[:, lb],
                                start=True, stop=True,
                                skip_group_check=True)
                            nc.scalar.copy(out=nc_sb[:, lb], in_=ncv)
                        lq = slice(bp * 1024, (bp + 1) * 1024)
                        nc.sync.dma_start(out=o_nc[:, lq], in_=nc_sb[:, lq])


def build():
    nc = bacc.Bacc("TRN2", target_bir_lowering=False, debug=False)
    qt2 = nc.dram_tensor("qt2", [C, L], BF16, kind="ExternalInput").ap()
    kt = nc.dram_tensor("kt", [E, 2, L], BF16, kind="ExternalInput").ap()
    ks = nc.dram_tensor("ks", [C, NCH, C], BF16, kind="ExternalInput").ap()
    v2m = nc.dram_tensor("v2m", [C, NCH, C], BF16, kind="ExternalInput").ap()
    o_vc = nc.dram_tensor("o_vc", [C, NCH, C], BF16,
                          kind="ExternalOutput").ap()
    o_nc = nc.dram_tensor("o_nc", [C, L], BF16, kind="ExternalOutput").ap()
    with tile.TileContext(nc) as tc:
        emit(tc, nc, qt2, kt, ks, v2m, o_vc, o_nc)
    nc.compile()
    return nc


_NC = None
_last_in_maps = None


def _get_nc():
    global _NC
    if _NC is None:
        _NC = build()
    return _NC


def _bf16(x):
    import ml_dtypes
    return np.ascontiguousarray(x, dtype=np.float32).astype(ml_dtypes.bfloat16)


def _feat(x):
    # elu(x) + 1 in fp32: exp(min(x,0)) + relu(x)
    return np.exp(np.minimum(x, 0.0)) + np.maximum(x, 0.0)


def kernel(queries, keys, values, key_mask):
    global _last_in_maps
    nc = _get_nc()
    queries = np.asarray(queries, dtype=np.float32)
    keys = np.asarray(keys, dtype=np.float32)
    values = np.asarray(values, dtype=np.float32)
    key_mask = np.asarray(key_mask, dtype=np.float32)

    Qf = _feat(queries)
    Kf = _feat(keys) * key_mask[:, :, None, None]
    Vm = values * key_mask[:, :, None, None]
    denc = np.einsum('nlhe,nlhe->nlh', Qf, np.cumsum(Kf, axis=1)) + EPS
    dennc = np.einsum('nlhe,nhe->nlh', Qf, Kf.sum(axis=1)) + EPS

    in_maps = []
    for i in range(8):
        n, h0 = i // 4, 2 * (i % 4)
        qh = Qf[n, :, h0:h0 + 2, :]                   # [L, 2, 64]
        kh = Kf[n, :, h0:h0 + 2, :]
        vh = Vm[n, :, h0:h0 + 2, :]
        qs = qh.transpose(1, 2, 0).reshape(C, L)      # stacked heads
        in_maps.append({
            "qt2": _bf16(qs),
            "kt": _bf16(kh.transpose(2, 1, 0)),
            "ks": _bf16(kh.reshape(NCH, C, C).transpose(1, 0, 2)),
            "v2m": _bf16(vh.reshape(NCH, C, C).transpose(1, 0, 2)),
        })
    _last_in_maps = in_maps
    res = run_bass_kernel_spmd(nc, in_maps, core_ids=list(range(8)))
    V = np.empty((N, L, H, M), np.float32)
    Vc = np.empty((N, L, H, M), np.float32)
    for i in range(8):
        n, h0 = i // 4, 2 * (i % 4)
        ovc = res.results[i]["o_vc"].astype(np.float32)   # [128, NCH, 128]
        onc = res.results[i]["o_nc"].astype(np.float32)   # [128, L]
        num_c = ovc.transpose(1, 2, 0).reshape(L, C)      # [l, (h m)]
        num_n = onc.T                                     # [l, (h m)]
        for h in range(2):
            Vc[n, :, h0 + h, :] = (num_c[:, h * E:(h + 1) * E]
                                   / denc[n, :, h0 + h, None])
            V[n, :, h0 + h, :] = (num_n[:, h * E:(h + 1) * E]
                                  / dennc[n, :, h0 + h, None])
    return (V, Vc)
